# revision 5
# baseline (speedup 1.0000x reference)
"""CAF forward kernel v2 for 8 TRN2 NeuronCores.

Exploits gamma == 0 (cross-attention collapses; resnet branch dead;
refine conv1x1 on cat([es,es]) becomes W_eff = refine_w[:,:64]+refine_w[:,64:];
refine_b drops out entirely because train-mode BN is shift-invariant).

Sharding: core i = (batch i//2, image-row half i%2) with a 3-row halo.
No CC#1: each core also loads the partner half's rows and reduces them
locally for the channel-attention mean.  One AllReduce (CC#2) for BN stats.

Speed keys vs the original: exact-fp32 proj/refine kept off the additive
error path but run warm (PE p-state fillers), avg-map row fused into the
proj matmul as a 65th output row (fp32-exact s, gate-grade bf16 maps),
per-chunk pipelined max map, 7-DMA ky-merged im2col for the 7x7 conv,
fp32r/bf16 for everything that only feeds the multiplicative sigmoid
gates, PE ones-matmul broadcast of the spatial sigmoid, act-table loads
hidden under input DMA / CC#2, split Act/DVE PSUM drains, one collective
(CC#1 replaced by a local reduce over the partner half's rows).
"""

import numpy as np

EPS = 1e-5

B, CIN, H, W = 4, 128, 64, 64
C = 64            # projected channels
R = C // 16       # channel attention reduction
C2 = 2 * C        # refine output channels
NCORES = 8
HALO = 3
ROWS = 32                 # output rows per core
NR = ROWS + 2 * HALO      # input rows incl halo = 38
NF = NR * W               # free size of s = 2432
OFF = HALO * W            # offset of my rows in free dim = 192
NO = ROWS * W             # my output pixels = 2048
NPIX_ALL = B * H * W      # 16384
NEX = NO                  # partner-rows tile cols (zero-padded)

W70 = 70
MROW = NR * W70 + 28      # padded maps row incl slack = 2688

# f32 const blob columns
F_WEFF = 0                # [64, 128] w_effT
F_DRAINB = 128            # [65, 1] proj_b ++ avg_b
F_BNS = 129               # [128, 1]
F_BNB = 130               # [128, 1]
F_EPS = 131               # [128, 1]
F_SAB = 132               # [1, 1]
F_PBSC = 133              # [64, 1] proj_b * 1856/4096
F_CAPROJ = 134            # [128, 65] proj_wT ++ avg col (fp32)
F_CA1 = 199               # [64, 4] ca_w1T
F_CA2 = 203               # [4, 64] ca_w2T
F_EYE64 = 267             # [64, 64] fp32
NCOLF = 331
# bf16 const blob columns
B_EYE128 = 0              # [128, 128]
B_W98 = 128               # [98, 1]
NCOLB = 129

CH = 512
CSPLIT = [(0, 512), (512, 1024), (1024, 1536), (1536, 2048), (2048, NF)]

_cache = {}


def _build_program(use_cc=True):
    import concourse.bacc as bacc
    import concourse.bass as bass
    import concourse.tile as tile
    from concourse import mybir

    fp32 = mybir.dt.float32
    fp32r = mybir.dt.float32r
    bf16 = mybir.dt.bfloat16
    AF = mybir.ActivationFunctionType
    ALU = mybir.AluOpType
    AX = mybir.AxisListType

    nc = bacc.Bacc(
        "TRN2",
        target_bir_lowering=False,
        debug=False,
        enable_asserts=True,
        num_devices=NCORES,
    )

    x_d = nc.dram_tensor("x", [CIN, NF], fp32, kind="ExternalInput").ap()
    xe_d = nc.dram_tensor("xe", [CIN, NEX], fp32, kind="ExternalInput").ap()
    cf_d = nc.dram_tensor("constf", [CIN, NCOLF], fp32, kind="ExternalInput").ap()
    cb_d = nc.dram_tensor("constb", [CIN, NCOLB], bf16, kind="ExternalInput").ap()
    ones_d = nc.dram_tensor("onesr", [1, CIN], fp32r, kind="ExternalInput").ap()
    out_d = nc.dram_tensor("out", [C2, NO], fp32, kind="ExternalOutput").ap()

    with tile.TileContext(nc) as tc:
        with (
            tc.tile_pool(name="consts", bufs=1) as consts,
            tc.tile_pool(name="work", bufs=1) as work,
            tc.tile_pool(name="psum", bufs=2, space="PSUM") as psum,
            tc.tile_pool(name="psy", bufs=4, space="PSUM") as psy,
            tc.tile_pool(name="dram", bufs=1, space="DRAM") as dram,
        ):
            # ---- first input chunk goes ahead of everything ----
            x_sb = work.tile([CIN, NF], fp32)
            nc.sync.dma_start(out=x_sb[:, 0:CH], in_=x_d[:, 0:CH])
            # ---- constants (Act queue so SP keeps streaming x) ----
            cf = consts.tile([CIN, NCOLF], fp32)
            nc.scalar.dma_start(out=cf, in_=cf_d)
            cb = consts.tile([CIN, NCOLB], bf16)
            nc.scalar.dma_start(out=cb, in_=cb_d)
            ones128 = consts.tile([1, CIN], fp32r, name="ones128")
            nc.gpsimd.dma_start(out=ones128, in_=ones_d)
            projx = cf[:, F_CAPROJ:F_CAPROJ + C + 1]      # [128,65] fp32
            ca_w1T = cf[0:C, F_CA1:F_CA1 + R]
            ca_w2T = cf[0:R, F_CA2:F_CA2 + C]
            ca_projT = cf[:, F_CAPROJ:F_CAPROJ + C]
            eye64 = cf[0:C, F_EYE64:F_EYE64 + C]
            w_effT = cf[0:C, F_WEFF:F_WEFF + C2]
            drainb = cf[0:C + 1, F_DRAINB:F_DRAINB + 1]
            bn_s = cf[:, F_BNS:F_BNS + 1]
            bn_b = cf[:, F_BNB:F_BNB + 1]
            eps_sb = cf[:, F_EPS:F_EPS + 1]
            sa_b = cf[0:1, F_SAB:F_SAB + 1]
            pbsc = cf[0:C, F_PBSC:F_PBSC + 1]
            eye128 = cb[:, B_EYE128:B_EYE128 + CIN]
            w98 = cb[0:98, B_W98:B_W98 + 1]

            # warm the sigmoid table set (covers identity/copy/relu/square
            # too) while the input DMA streams
            # PE p-state warm-up: dep-free filler matmuls keep the Tensor
            # engine busy through the input DMA so the first real (fp32)
            # matmuls run at full clock instead of the cold 1/3.7 rate
            ones_bf = work.tile([1, CH], bf16, name="ones_bf")
            nc.vector.memset(ones_bf, 1.0)
            warm = work.tile([1, 4], fp32, name="warm")
            nc.vector.memset(warm, 1.0)
            nc.scalar.activation(out=warm, in_=warm, func=AF.Sigmoid)
            for fi in range(7):
                fpad = psum.tile([C, CH], fp32, tag="mm", name=f"fill{fi}")
                nc.tensor.matmul(fpad, ones_bf[0:1, 0:C], ones_bf,
                                 start=True, stop=True)

            # ---- remaining input DMAs ----
            for ic, (c0, c1) in enumerate(CSPLIT[1:]):
                nc.sync.dma_start(out=x_sb[:, c0:c1], in_=x_d[:, c0:c1])
            xe_sb = work.tile([CIN, NEX], fp32)
            for ic in range(4):
                c0, c1 = ic * CH, (ic + 1) * CH
                nc.sync.dma_start(out=xe_sb[:, c0:c1], in_=xe_d[:, c0:c1])
            x_r = x_sb

            # maps zero-init on the (otherwise idle) gpsimd engine
            maps = work.tile([2, MROW], bf16)
            nc.gpsimd.memset(maps, 0.0)
            mp_r = maps[:, 0:NR * W70].rearrange("p (y c) -> p y c", c=W70)

            # ---- proj(+avg row) matmul, drains, max-map transposes ----
            sxa = work.tile([C + 1, NF], fp32)
            sxa_r = sxa
            sxa_f = sxa
            ca_acc = work.tile([C + 1, 10], fp32)
            nc.vector.memset(ca_acc, 0.0)
            NK = NF // CIN  # 19 pixel blocks of 128
            mx_t = work.tile([CIN, NK], bf16)
            s_psums = []
            for ic, (c0, c1) in enumerate(CSPLIT):
                s_psum = psum.tile([C + 1, c1 - c0], fp32, tag="mm",
                                   name=f"sp{ic}")
                nc.tensor.matmul(s_psum, projx, x_r[:, c0:c1],
                                 start=True, stop=True)
                s_psums.append(s_psum)
                # drain + bias + channel-attention accumulation over the
                # accum window [OFF, NF) (tile rows 3..37)
                mid = (c0 + c1) // 2
                segs = [(c0, mid, nc.scalar, 0), (mid, c1, nc.vector, 1)]
                for a, b2, eng, half in segs:
                    subs = ([(a, b2)] if a >= OFF or b2 <= OFF
                            else [(a, OFF), (OFF, b2)])
                    for a2, b3 in subs:
                        acc = (ca_acc[:, 2 * ic + half:2 * ic + half + 1]
                               if a2 >= OFF else None)
                        if eng is nc.scalar:
                            nc.scalar.activation(
                                out=sxa[:, a2:b3],
                                in_=s_psum[:, a2 - c0:b3 - c0],
                                func=AF.Identity, bias=drainb, scale=1.0,
                                accum_out=acc)
                        else:
                            nc.vector.tensor_scalar(
                                out=sxa[:, a2:b3],
                                in0=s_psum[:, a2 - c0:b3 - c0],
                                scalar1=drainb, scalar2=0.0,
                                op0=ALU.add, op1=ALU.add, accum_out=acc)
                # avg-map row -> maps row 0 (bf16, pitch 70) on gpsimd
                r_lo, r_hi = c0 // W, c1 // W
                nc.gpsimd.tensor_copy(
                    out=mp_r[0:1, r_lo:r_hi, HALO:HALO + W],
                    in_=sxa_f[C:C + 1, c0:c1].rearrange("p (y c) -> p y c", c=W))
                # max map: PE-transpose 128-pixel blocks, DVE reduce
                nk0, nk1 = c0 // CIN, c1 // CIN
                tp = psum.tile([CIN, (nk1 - nk0) * C], fp32, tag="aux",
                               name=f"tp{ic}")
                for k in range(nk1 - nk0):
                    nc.tensor.transpose(
                        tp[:, k * C:(k + 1) * C],
                        sxa_r[0:C, (nk0 + k) * CIN:(nk0 + k + 1) * CIN],
                        eye64)
                nc.vector.reduce_max(
                    out=mx_t[:, nk0:nk1],
                    in_=tp.rearrange("p (k c) -> p k c", c=C),
                    axis=AX.X)

            # max map tail: transpose back and DMA into maps row 1
            mxb = psum.tile([NK, CIN], bf16, tag="aux", name="mxb")
            nc.tensor.transpose(mxb, mx_t, eye128)
            mx_row = work.tile([NK, CIN], bf16)
            nc.vector.tensor_copy(out=mx_row, in_=mxb)
            nc.sync.dma_start(out=mp_r[1:2, 0:NR, HALO:HALO + W], in_=mx_row)

            # ---- im2col in one hop: 7 ky-merged window DMAs (both ci per
            # DMA; w98 row order ky*14 + ci*7 + kx) ----
            m98 = work.tile([98, ROWS * W70], bf16)
            IENG = [nc.sync, nc.scalar, nc.gpsimd, nc.sync, nc.scalar,
                    nc.gpsimd, nc.sync]
            for ky in range(7):
                srcw = bass.AP(
                    tensor=maps[0:1, :].tensor, offset=ky * W70,
                    ap=[[MROW, 2], [1, 7], [1, ROWS * W70]])
                IENG[ky].dma_start(
                    out=m98[ky * 14:(ky + 1) * 14, :].rearrange(
                        "p (o f) -> p o f", o=1),
                    in_=srcw)
            m98_r = m98.rearrange("p (y c) -> p y c", c=W70)

            # ---- partner-rows sums for channel attention (replaces CC#1) --
            xs_acc = work.tile([CIN, 4], fp32)
            nc.vector.memset(xs_acc, 0.0)
            xtrash = work.tile([CIN, CH], fp32, name="xtrash")
            for ic in range(4):
                c0, c1 = ic * CH, (ic + 1) * CH
                if ic < 2:
                    nc.scalar.activation(
                        out=xtrash, in_=xe_sb[:, c0:c1], func=AF.Copy,
                        accum_out=xs_acc[:, ic:ic + 1])
                else:
                    nc.vector.reduce_sum(
                        out=xs_acc[:, ic:ic + 1], in_=xe_sb[:, c0:c1],
                        axis=AX.X)
            xsum = work.tile([CIN, 1], fp32)
            nc.vector.reduce_sum(out=xsum, in_=xs_acc, axis=AX.X)

            # ---- channel attention scalars ----
            ca_red = work.tile([C + 1, 1], fp32)
            nc.vector.reduce_sum(out=ca_red, in_=ca_acc, axis=AX.X)
            mm_ca = psum.tile([C, 1], fp32, tag="aux", name="mmca")
            nc.tensor.matmul(mm_ca, ca_projT, xsum, start=True, stop=True)
            t1 = work.tile([C, 1], fp32)
            nc.vector.tensor_scalar(
                out=t1, in0=ca_red[0:C, :], scalar1=1.0 / NPIX_ALL * NCORES / 2,
                scalar2=pbsc, op0=ALU.mult, op1=ALU.add)
            mean_s = work.tile([C, 1], fp32)
            nc.vector.scalar_tensor_tensor(
                out=mean_s, in0=mm_ca, scalar=1.0 / NPIX_ALL * NCORES / 2,
                in1=t1, op0=ALU.mult, op1=ALU.add)
            h_ps = psum.tile([R, 1], fp32, tag="aux", name="hps")
            nc.tensor.matmul(h_ps, ca_w1T, mean_s, start=True, stop=True)
            h_sb = work.tile([R, 1], fp32)
            nc.scalar.activation(out=h_sb, in_=h_ps, func=AF.Relu)
            scl_ps = psum.tile([C, 1], fp32, tag="aux", name="sclps")
            nc.tensor.matmul(scl_ps, ca_w2T, h_sb, start=True, stop=True)
            scl = work.tile([C, 1], fp32)
            nc.scalar.activation(out=scl, in_=scl_ps, func=AF.Sigmoid)
            w_scl = work.tile([C, C2], fp32)
            nc.vector.tensor_scalar_mul(w_scl, w_effT, scl)
            w_scl_r = w_scl

            # ---- conv + sigmoid + bcast64 + mul-into-s + refine + stats --
            # sg broadcasts sigma to 64 partitions; es = s * sigma stays the
            # only SBUF materialization; y lives in PSUM until the final
            # normalize (TensorTensor may read at most one PSUM operand).
            for fi in range(2):
                fpad = psum.tile([C, CH], fp32, tag="aux", name=f"lfill{fi}")
                nc.tensor.matmul(fpad, ones_bf[0:1, 0:C], ones_bf,
                                 start=True, stop=True)
            for fi in range(2):
                fpad = psum.tile([C, CH], fp32, tag="aux", name=f"gfill{fi}")
                nc.tensor.matmul(fpad, ones_bf[0:1, 0:C],
                                 m98[0:1, fi * CH:(fi + 1) * CH],
                                 start=True, stop=True)
            sig_row = work.tile([1, NO], fp32r)
            sig_r = sig_row
            ones64 = ones128[0:1, 0:C]
            es = work.tile([C, NO], fp32)
            bst = work.tile([C2, 4, 6], fp32)
            for iq in range(4):
                c0 = iq * CH
                r0 = iq * 8
                cv = psum.tile([1, CH], fp32, tag="aux", name=f"cv{iq}")
                nc.tensor.matmul(cv, w98, m98_r[:, r0:r0 + 8, 0:W],
                                 start=True, stop=True)
                nc.scalar.activation(
                    out=sig_row[0:1, c0:c0 + CH], in_=cv,
                    func=AF.Sigmoid, bias=sa_b, scale=1.0)
                sg = psum.tile([C, CH], fp32, tag="mm", name=f"sg{iq}")
                nc.tensor.matmul(sg, ones64, sig_r[0:1, c0:c0 + CH],
                                 start=True, stop=True)
                nc.vector.tensor_tensor(out=es[:, c0:c0 + CH],
                                        in0=sxa_f[0:C, OFF + c0:OFF + c0 + CH],
                                        in1=sg, op=ALU.mult)
            yps = []
            for iq in range(4):
                c0 = iq * CH
                yp = psy.tile([C2, CH], fp32, tag="y", name=f"yp{iq}")
                nc.tensor.matmul(yp, w_scl_r, es[:, c0:c0 + CH],
                                 start=True, stop=True)
                yps.append(yp)
                nc.vector.bn_stats(out=bst[:, iq, :], in_=yp)

            # load the sqrt table set while CC#2 is in flight (input dep on
            # sig_row keeps the scheduler from hoisting it early)
            nc.scalar.activation(
                out=warm, in_=sig_row.bitcast(fp32)[0:1, NO - 4:NO],
                func=AF.Sqrt)

            # local BN sums
            bn_mv = work.tile([C2, 2], fp32)
            nc.vector.bn_aggr(out=bn_mv, in_=bst)
            sum_y = work.tile([C2, 1], fp32)
            nc.vector.tensor_scalar_mul(sum_y, bn_mv[:, 0:1], float(NO))
            msq_l = work.tile([C2, 1], fp32)
            nc.vector.tensor_mul(msq_l, bn_mv[:, 0:1], bn_mv[:, 0:1])
            sum_y2 = work.tile([C2, 1], fp32)
            nc.vector.tensor_add(sum_y2, bn_mv[:, 1:2], msq_l)
            nc.vector.tensor_scalar_mul(sum_y2, sum_y2, float(NO))

            # ---- CC#2: global BN stats ----
            gsum = work.tile([C2, 1], fp32)
            gsq = work.tile([C2, 1], fp32)
            if use_cc:
                cc2_in = dram.tile([2, C2], fp32)
                cc2_out = dram.tile([2, C2], fp32)
                nc.sync.dma_start(out=cc2_in[0:1, :], in_=sum_y)
                nc.sync.dma_start(out=cc2_in[1:2, :], in_=sum_y2)
                nc.gpsimd.collective_compute(
                    "AllReduce", mybir.AluOpType.add,
                    replica_groups=[[0, 1, 2, 3, 4, 5, 6, 7]],
                    ins=[cc2_in.opt()], outs=[cc2_out.opt()],
                )
                nc.sync.dma_start(out=gsum, in_=cc2_out[0:1, :])
                nc.sync.dma_start(out=gsq, in_=cc2_out[1:2, :])
            else:
                nc.vector.tensor_scalar_mul(gsum, sum_y, float(NCORES))
                nc.vector.tensor_scalar_mul(gsq, sum_y2, float(NCORES))

            # BN coeffs: a = bn_s * rsqrt(var+eps); b = bn_b - mean*a
            mean = work.tile([C2, 1], fp32)
            nc.vector.tensor_scalar_mul(mean, gsum, 1.0 / NPIX_ALL)
            msq = work.tile([C2, 1], fp32)
            nc.vector.tensor_mul(msq, mean, mean)
            var = work.tile([C2, 1], fp32)
            nc.vector.tensor_scalar(
                out=var, in0=gsq, scalar1=1.0 / NPIX_ALL, scalar2=0.0,
                op0=ALU.mult, op1=ALU.add)
            nc.vector.tensor_sub(var, var, msq)
            std = work.tile([C2, 1], fp32)
            nc.scalar.activation(out=std, in_=var, func=AF.Sqrt,
                                 bias=eps_sb, scale=1.0)
            rstd = work.tile([C2, 1], fp32)
            nc.vector.reciprocal(rstd, std)
            a_co = work.tile([C2, 1], fp32)
            nc.vector.tensor_mul(a_co, rstd, bn_s)
            b_co = work.tile([C2, 1], fp32)
            nc.vector.tensor_mul(b_co, mean, a_co)
            nc.vector.tensor_sub(b_co, bn_b, b_co)

            # ---- final normalize + relu + store ----
            out_sb = work.tile([C2, NO], fp32)
            for iq in range(4):
                c0 = iq * CH
                if iq % 2 == 0:
                    nc.scalar.activation(
                        out=out_sb[:, c0:c0 + CH], in_=yps[iq],
                        func=AF.Relu, bias=b_co, scale=a_co)
                else:
                    nc.vector.tensor_scalar(
                        out=out_sb[:, c0:c0 + CH], in0=yps[iq],
                        scalar1=a_co, scalar2=b_co, op0=ALU.mult, op1=ALU.add)
                    nc.vector.tensor_relu(out=out_sb[:, c0:c0 + CH],
                                          in_=out_sb[:, c0:c0 + CH])
                nc.sync.dma_start(out=out_d[:, c0:c0 + CH],
                                   in_=out_sb[:, c0:c0 + CH])

    nc.compile()
    return nc


def _host_prep(inputs):
    """Build the 8 per-core input maps."""
    import ml_dtypes

    swin = np.ascontiguousarray(np.asarray(inputs["swin_feat"], np.float32))
    proj_w = np.asarray(inputs["proj_w"], np.float32)
    proj_b = np.asarray(inputs["proj_b"], np.float32)
    refine_w = np.asarray(inputs["refine_w"], np.float32)
    sa_w = np.asarray(inputs["sa_w"], np.float32)

    w_eff = refine_w[:, :C] + refine_w[:, C:]
    # row order r = ky*14 + ci*7 + kx; the avg map row already holds the
    # channel MEAN (v_sum has the /64 baked in), so no tap scaling here
    w98 = np.ascontiguousarray(sa_w[0].transpose(1, 0, 2)).reshape(98)

    cf = np.zeros((CIN, NCOLF), np.float32)
    cf[0:C, F_WEFF:F_WEFF + C2] = w_eff.T
    cf[0:C, F_DRAINB] = proj_b
    cf[C, F_DRAINB] = proj_b.sum() / C
    cf[:, F_BNS] = np.asarray(inputs["bn_scale"], np.float32)
    cf[:, F_BNB] = np.asarray(inputs["bn_bias"], np.float32)
    cf[:, F_EPS] = EPS
    cf[0, F_SAB] = float(np.asarray(inputs["sa_b"]).reshape(-1)[0])
    cf[0:C, F_PBSC] = proj_b * ((H * W - 35 * W) / (H * W))
    cf[:, F_CAPROJ:F_CAPROJ + C] = proj_w.T
    cf[:, F_CAPROJ + C] = proj_w.sum(axis=0) / C
    cf[0:C, F_EYE64:F_EYE64 + C] = np.eye(C)
    cf[0:C, F_CA1:F_CA1 + R] = np.asarray(inputs["ca_w1"], np.float32).T
    cf[0:R, F_CA2:F_CA2 + C] = np.asarray(inputs["ca_w2"], np.float32).T


    cb = np.zeros((CIN, NCOLB), np.float32)
    cb[:, B_EYE128:B_EYE128 + CIN] = np.eye(CIN)
    cb[0:98, B_W98] = w98
    cb = cb.astype(ml_dtypes.bfloat16)

    in_maps = []
    for i in range(NCORES):
        b, h = divmod(i, 2)
        r0 = 32 * h - HALO
        xpad = np.zeros((CIN, NR, W), np.float32)
        lo, hi = max(r0, 0), min(r0 + NR, H)
        xpad[:, lo - r0:hi - r0, :] = swin[b, :, lo:hi, :]
        # partner rows not covered by the accum window (tile rows 3..37)
        xe = np.zeros((CIN, ROWS, W), np.float32)
        if h == 0:
            xe[:, 0:H - 35, :] = swin[b, :, 35:H, :]     # rows 35..63
        else:
            xe[:, :, :] = swin[b, :, 0:ROWS, :]          # rows 0..31
        in_maps.append({
            "x": xpad.reshape(CIN, NF),
            "xe": xe.reshape(CIN, NEX),
            "constf": cf, "constb": cb,
            "onesr": np.ones((1, CIN), np.float32),
        })
    return in_maps


def _reference_numpy(inputs):
    """Exact numpy replica of the reference (fallback for gamma != 0)."""
    f = lambda k: np.asarray(inputs[k], np.float64)
    swin, resnet = f("swin_feat"), f("resnet_feat")
    proj_w, proj_b = f("proj_w"), f("proj_b")
    ca_w1, ca_w2 = f("ca_w1"), f("ca_w2")
    sa_w, sa_b = f("sa_w"), f("sa_b")
    q_w, q_b, k_w, k_b = f("q_w"), f("q_b"), f("k_w"), f("k_b")
    v_w, v_b, gamma = f("v_w"), f("v_b"), f("gamma")
    refine_w, refine_b = f("refine_w"), f("refine_b")
    bn_scale, bn_bias = f("bn_scale"), f("bn_bias")

    def conv1x1(x, w, b=None):
        y = np.einsum("bchw,oc->bohw", x, w)
        if b is not None:
            y = y + b[None, :, None, None]
        return y

    def channel_attention(x):
        avg = x.mean(axis=(2, 3))
        hh = np.maximum(avg @ ca_w1.T, 0)
        s = 1 / (1 + np.exp(-(hh @ ca_w2.T)))
        return s[:, :, None, None]

    def spatial_attention(x):
        avg = x.mean(axis=1, keepdims=True)
        mx = x.max(axis=1, keepdims=True)
        cat = np.concatenate([avg, mx], axis=1)
        bsz = x.shape[0]
        y = np.zeros((bsz, 1, H, W))
        pad = np.zeros((bsz, 2, H + 6, W + 6))
        pad[:, :, 3:-3, 3:-3] = cat
        for ky in range(7):
            for kx in range(7):
                for ci in range(2):
                    y[:, 0] += sa_w[0, ci, ky, kx] * pad[:, ci, ky:ky + H, kx:kx + W]
        return 1 / (1 + np.exp(-(y + sa_b[None, :, None, None])))

    def cross_attention(x, y):
        bsz = x.shape[0]
        q = conv1x1(x, q_w, q_b).reshape(bsz, -1, H * W)
        k = conv1x1(y, k_w, k_b).reshape(bsz, -1, H * W)
        v = conv1x1(y, v_w, v_b).reshape(bsz, C, H * W)
        att = np.einsum("bcn,bcm->bnm", q, k)
        att = att - att.max(axis=-1, keepdims=True)
        att = np.exp(att)
        att /= att.sum(axis=-1, keepdims=True)
        out = np.einsum("bcm,bnm->bcn", v, att).reshape(bsz, C, H, W)
        return gamma * out + x

    s = conv1x1(swin, proj_w, proj_b)
    r = conv1x1(resnet, proj_w, proj_b)
    es = s * channel_attention(s) * spatial_attention(s)
    er = r * channel_attention(r) * spatial_attention(r)
    cross = cross_attention(es, er)
    cat = np.concatenate([cross, es], axis=1)
    y = conv1x1(cat, refine_w, refine_b)
    mean = y.mean(axis=(0, 2, 3), keepdims=True)
    var = y.var(axis=(0, 2, 3), keepdims=True)
    xn = (y - mean) / np.sqrt(var + EPS)
    out = np.maximum(xn * bn_scale[None, :, None, None] + bn_bias[None, :, None, None], 0)
    return out.astype(np.float32)


def kernel(**inputs):
    gamma = np.asarray(inputs["gamma"])
    if np.any(gamma != 0):
        return _reference_numpy(inputs)

    from concourse import bass_utils

    if "nc" not in _cache:
        _cache["nc"] = _build_program()
    nc = _cache["nc"]

    in_maps = _host_prep(inputs)
    res = bass_utils.run_bass_kernel_spmd(nc, in_maps, core_ids=list(range(NCORES)))

    out = np.empty((B, C2, H, W), np.float32)
    for i in range(NCORES):
        b, h = divmod(i, 2)
        out[b, :, 32 * h:32 * h + 32, :] = res.results[i]["out"].reshape(C2, 32, W)
    return out
